# revision 59
# baseline (speedup 1.0000x reference)
"""Memory-augmented forecaster kernel for 8 Trainium2 NeuronCores.

Pipeline (3 SPMD launches; host does only sharding/layout/merge between):
  All hidden-state traffic uses a host-transposed layout hsT[b] = (D, S):
  the per-batch delta broadcast becomes a per-partition-scalar add and the
  S-mean becomes a free-axis reduction, so both split across the DVE, ACT
  and Pool engines with no PE broadcast matmuls and no staging.

  L1 (batch-sharded, 32 queries/core, ~55us): sums[b, :] = sum_S hsT[b]
      via free-axis reductions interleaved over DVE/ACT/Pool (Pool adds
      the two S-halves, DVE finishes), hidden under the bf16 hidden-state
      read (DMA-bound, 47us floor).  The last two batches are split
      per-dj across engines to shorten the tail.  Host divides by S.
  L2 (bank-sharded, 12500 rows/core padded to 12504, ~37us): sims =
      q @ bankT as an fp8e4m3 DoubleRow PE matmul (256-deep contraction,
      0.5 cyc/col, fp32 PSUM; inputs prescaled by 64 so the fp8 grid
      centers).  Per supertile (up to 1024 cols of DMA tiles in one 4-bank
      PSUM buffer) the ACT engine stages PSUM to f16 once and the DVE computes
      per-8-col group maxima as three packed halving rounds (2x mode;
      group g = stride-sw/8 column set).  A chained running-top-8
      (Max8/MaxIndex8 after supertiles 3/6/9/10) folds finished regions
      under the sweep so the final scan stays short; the host unwinds the
      position chain.  The device output is selection-only: the host
      rescans the 8 cores x 8 groups
      x 8 cols = 512 candidate columns per query exactly in f32 (so fp8
      noise, ~1.3e-3 on sims, never reaches the result), with a
      per-query exact-recompute fallback if the provable sufficiency
      bound fails (never fires on random data).
  L3 (batch-sharded, ~104us): gated cross-attention over the top-16
      memories (weighted-sum pushed before the Wv projection), gating,
      LayerNorm; delta = LN(fused) - series is PE-transposed onto
      partitions and added to hsT as a per-partition scalar on
      DVE/ACT/Pool.  Wv|Wo ship fp8e4m3 (prescaled by 64; ACT
      upconverts to bf16 on device, adding only benign weight
      quantization noise -- Wq/Wk stay bf16 since score noise flips
      attention near-ties) and bq|bk is host-packed to the device layout.
      Output is written bf16 (halves the write traffic; rel-err ~5.4e-3
      vs the 2e-2 budget) and the host transposes/upcasts back to
      (B, S, D) f32.  DMA-bound at ~100us busy of its traffic floor.
"""

import os
import numpy as np

import concourse.bacc as bacc
import concourse.mybir as mybir
from concourse import bass_utils
from concourse.tile import TileContext
from concourse.masks import make_identity

F32 = mybir.dt.float32
F16 = mybir.dt.float16
BF16 = mybir.dt.bfloat16
F8E4 = mybir.dt.float8e4
U16 = mybir.dt.uint16
AX = mybir.AxisListType
OP = mybir.AluOpType
ACT = mybir.ActivationFunctionType

NP_BF16 = mybir.dt.np(BF16)
NP_F8E4 = mybir.dt.np(F8E4)
Q8_SCALE = 64.0       # power-of-2 prescale centering fp8e4m3's range

B, S, D = 256, 512, 512
M, TOPK = 100000, 16
NC = 8
BL = B // NC          # 32 queries per core (L1/L3)
ML = M // NC          # 12500 bank rows per core (L2)
MLP = 12504           # padded to a multiple of 8 (4 zero columns)
G = 8                 # L2 group width for the PSUM group-max
NG = MLP // G         # 1563 groups per shard
CT = 512              # L2 DMA column tile
# L2 supertiles: column tiles sharing one PSUM buffer so the ACT
# f16-staging and DVE halving rounds run as wide ops.  The first supertile
# is a single narrow tile (earliest possible first stage) and the last is
# the 216-col remainder (shortest possible tail chain).
L2_TILES = [128, 384] + [CT] * 23 + [216]
assert sum(L2_TILES) == MLP and all(w % G == 0 for w in L2_TILES)
L2_C0 = [sum(L2_TILES[:i]) for i in range(len(L2_TILES))]
L2_ST = [[0], [1, 2]] + [[3 + 2 * k, 4 + 2 * k] for k in range(11)] + [[25]]
L2_SUPER = [sum(L2_TILES[t] for t in ts) for ts in L2_ST]
assert sum(L2_SUPER) == MLP
KJ = D // 128         # 4 contraction subtiles
# L2 chained running-top-8: after each listed supertile a Max8/MaxIndex8
# folds the finished region (plus the previous run-8) into a new run-8, so
# every fold is short and hides in the sweep; the final scan after the
# last supertile covers only [run8 | 27 tail groups].  gmx regions are
# separated by 8-slot run buffers so every scan is contiguous.
L2_STAGES = [3, 6, 9, 10]               # supertile index each fold follows
_SG = [0, 384, 768, 1152, 1280, NG]     # group-range region boundaries
GPAD = 5                                # pad groups after the tail region


def _group_cols_lut():
    """Padded-shard columns of each group id: group g of an sw-wide
    supertile at s0 covers {s0 + (g - s0/8) + (sw/8)*i} (the device's
    halving rounds pair stride-sw/8 columns)."""
    lut = np.zeros((NG, G), np.int64)
    s0 = 0
    for sw in L2_SUPER:
        g0, ng = s0 // G, sw // G
        loc = np.arange(ng)
        lut[g0:g0 + ng] = s0 + loc[:, None] + ng * np.arange(G)[None, :]
        s0 += sw
    return lut


GROUP_COLS = _group_cols_lut()
SCALE = D ** -0.5
LN_EPS = 1e-5
GATE_TEMP = 1.0
THRESH = 0.0
NEG = -1.0e38
SUFF_MARGIN = 1.2e-2  # device fp8/f16 vs host-f32 sim slack (~9 sigma)

EXEC_NS = {}

_programs = {}


# ---------------------------------------------------------------- L1 -----
def _build_l1():
    nc = bacc.Bacc("TRN2", target_bir_lowering=False, debug=False)
    hsT = nc.dram_tensor("hsT", (BL, D, S), BF16, kind="ExternalInput").ap()
    sums_o = nc.dram_tensor("sums", (D, BL), F32, kind="ExternalOutput").ap()

    # Spread the per-batch S-reduction across DVE/ACT/Pool so every engine
    # stays under the DMA stream (~47us).  gpsimd tensor_reduce only does
    # partition-axis reductions, so Pool instead adds the two S-halves
    # and DVE finishes the half-width reduce.  Engines must be interleaved
    # in program order (per-engine queues execute in order); the final two
    # batches are split per-dj across engines so the tail after the last
    # DMA is ~1.3us instead of a full batch reduction.
    sched = list("vapvpavp" * 4)
    sched[30] = sched[31] = "s"

    with TileContext(nc) as tc:
        with (
            tc.tile_pool(name="hidp", bufs=20) as hidp,
            tc.tile_pool(name="sml", bufs=1) as sml,
            tc.tile_pool(name="scr", bufs=3) as scrp,
        ):
            sm = sml.tile([128, KJ, BL], F32)
            # two batches per DMA: halves the per-DMA issue/semaphore
            # overhead on the sync queue
            tiles = {}
            for b0 in range(0, BL, 2):
                tt = hidp.tile([128, 2, KJ, S], BF16, tag="hload")
                nc.sync.dma_start(
                    tt[:, :, :, :],
                    hsT[b0:b0 + 2].rearrange("b (j p) s -> p b j s", p=128))
                tiles[b0] = tt
            for b in range(BL):
                t = tiles[b - b % 2][:, b % 2]
                eng = sched[b]
                if eng == "v":
                    nc.vector.tensor_reduce(
                        sm[:, :, b], t[:, :, :], axis=AX.X, op=OP.add)
                elif eng == "p":
                    # f32 halves tile: exact, and keeps DVE's share small
                    half = scrp.tile([128, KJ, S // 2], F32, tag="phalf")
                    nc.gpsimd.tensor_add(
                        half[:, :, :], t[:, :, :S // 2], t[:, :, S // 2:])
                    nc.vector.tensor_reduce(
                        sm[:, :, b], half[:, :, :], axis=AX.X, op=OP.add)
                elif eng == "a":
                    for dj in range(KJ):
                        scr = scrp.tile([128, S], F32, tag="ascr")
                        nc.scalar.activation(
                            scr[:, :], t[:, dj, :], ACT.Copy,
                            accum_out=sm[:, dj, b:b + 1])
                else:  # split: one dj per engine, all in parallel
                    nc.vector.tensor_reduce(
                        sm[:, 0:1, b], t[:, 0:1, :], axis=AX.X, op=OP.add)
                    scr = scrp.tile([128, S], F32, tag="ascr")
                    nc.scalar.activation(
                        scr[:, :], t[:, 1, :], ACT.Copy,
                        accum_out=sm[:, 1, b:b + 1])
                    half = scrp.tile([128, 1, S // 2], F32, tag="phalf")
                    nc.gpsimd.tensor_add(
                        half[:, :, :], t[:, 2:3, :S // 2], t[:, 2:3, S // 2:])
                    nc.vector.tensor_reduce(
                        sm[:, 2:3, b], half[:, :, :], axis=AX.X, op=OP.add)
                    nc.vector.tensor_reduce(
                        sm[:, 3:4, b], t[:, 3:4, :], axis=AX.X, op=OP.add)
            nc.sync.dma_start(
                sums_o.rearrange("(j p) b -> p j b", p=128), sm[:, :, :])
    nc.compile()
    return nc


# ---------------------------------------------------------------- L2 -----
def _build_l2():
    nc = bacc.Bacc("TRN2", target_bir_lowering=False, debug=False)
    # fp8e4m3 inputs (prescaled by Q8_SCALE on host): the PE runs DoubleRow
    # (256-deep contraction, 0.5 cyc/col) and the bank read halves.  Device
    # sims are selection-only -- the host rescans winning groups in f32 --
    # so the ~1.3e-3 fp8 sim noise only widens the sufficiency margin.
    qT = nc.dram_tensor("qT", (D, B), F8E4, kind="ExternalInput").ap()
    bankT = nc.dram_tensor("bankT", (D, MLP), F8E4, kind="ExternalInput").ap()
    # per query: 8 group-max values (f16) then 4 position octets (u16):
    # final, then stage2/1/0 -- host unwinds the run-8 chain
    tv_o = nc.dram_tensor("tv", (B, 48), F16, kind="ExternalOutput").ap()

    with TileContext(nc) as tc:
        with (
            tc.tile_pool(name="qp", bufs=1) as qp,
            tc.tile_pool(name="bkp", bufs=10) as bkp,
            tc.tile_pool(name="stg", bufs=3) as stg,
            tc.tile_pool(name="outp", bufs=1) as outp,
            tc.tile_pool(name="ps", bufs=2, space="PSUM") as psp,
        ):
            qt = qp.tile([128, 2, 2, B], F8E4)
            nc.sync.dma_start(
                qt[:, :, :, :],
                qT.rearrange("(j i p) b -> p j i b", p=128, i=2))
            # group maxima with an 8-slot run buffer after each region
            gx_w = NG + GPAD + 8 * len(L2_STAGES)
            gmx = outp.tile([128, 2, gx_w], F16)
            nc.vector.memset(gmx[:, :, gx_w - GPAD:], -60000.0)
            cand = outp.tile([128, 2, 48], F16)
            # gmx offset of group g (region r gets r*8 extra) and the
            # scan window [lo, hi) -> run-slot of each chain stage
            goff = lambda g: g + 8 * sum(1 for b in _SG[1:-1] if g >= b)
            stage_win = []
            lo = 0
            nwin = len(L2_STAGES) + 1
            for r in range(nwin):
                hi = goff(_SG[r + 1] - 1) + 1 if r < nwin - 1 else gx_w
                stage_win.append((lo, hi))
                lo = hi
            assert all(b - a <= 16384 for a, b in stage_win)
            bk_re = bankT.rearrange("(j i p) c -> p j i c", p=128, i=2)
            for sidx, sw in enumerate(L2_SUPER):
                s0 = sum(L2_SUPER[:sidx])
                g0 = s0 // G
                # per-blk stride padded to 1024 f32 so every matmul output
                # stays inside whole PSUM banks
                pt = psp.tile([128, 2, 1024], F32, tag="ps")
                off = 0
                for t in L2_ST[sidx]:
                    cw, c0 = L2_TILES[t], L2_C0[t]
                    bk = bkp.tile([128, 2, 2, CT], F8E4, tag="bk")
                    nc.sync.dma_start(
                        bk[:, :, :, :cw], bk_re[:, :, :, c0:c0 + cw])
                    for blk in range(2):
                        for j in range(2):
                            nc.tensor.matmul(
                                pt[:, blk, off:off + cw],
                                qt[:, j, :, blk * 128:(blk + 1) * 128],
                                bk[:, j, :, :cw],
                                start=(j == 0), stop=(j == 1),
                                perf_mode=mybir.MatmulPerfMode.DoubleRow,
                            )
                    off += cw
                # grouped max via one f16 staging op + 3 packed halving
                # rounds (DVE 2x mode; TensorReduce gets no 2x).  Group g of
                # this supertile covers columns {s0 + g + (sw/8)*i}.  The
                # last supertile runs per query block so block 0's final
                # top-8 overlaps block 1's staging.
                d0 = goff(g0)
                last = sidx == len(L2_SUPER) - 1
                for blk in ([0, 1] if last else [slice(None)]):
                    st = stg.tile([128, 2, 1024], F16, tag="st")
                    nc.scalar.copy(st[:, blk, :sw], pt[:, blk, :sw])
                    r1 = stg.tile([128, 2, 512], F16, tag="r1")
                    nc.vector.tensor_tensor(
                        r1[:, blk, :sw // 2], st[:, blk, :sw // 2],
                        st[:, blk, sw // 2:sw], op=OP.max)
                    r2 = stg.tile([128, 2, 256], F16, tag="r2")
                    nc.vector.tensor_tensor(
                        r2[:, blk, :sw // 4], r1[:, blk, :sw // 4],
                        r1[:, blk, sw // 4:sw // 2], op=OP.max)
                    nc.vector.tensor_tensor(
                        gmx[:, blk, d0:d0 + sw // G], r2[:, blk, :sw // 8],
                        r2[:, blk, sw // 8:sw // 4], op=OP.max)
                    if last:
                        lo, hi = stage_win[-1]
                        nc.vector.max(cand[:, blk, 0:8], gmx[:, blk, lo:hi])
                        nc.vector.max_index(
                            cand[:, blk, 8:16].bitcast(U16),
                            cand[:, blk, 0:8], gmx[:, blk, lo:hi])
                if sidx in L2_STAGES:
                    # fold the finished region (+ previous run-8) into a new
                    # run-8; each short scan hides under the remaining sweep
                    r = L2_STAGES.index(sidx)
                    lo, hi = stage_win[r]
                    nf = len(L2_STAGES)
                    for blk in range(2):
                        nc.vector.max(gmx[:, blk, hi:hi + 8],
                                      gmx[:, blk, lo:hi])
                        nc.vector.max_index(
                            cand[:, blk, 16 + 8 * (nf - 1 - r):
                                 24 + 8 * (nf - 1 - r)].bitcast(U16),
                            gmx[:, blk, hi:hi + 8], gmx[:, blk, lo:hi])
            nc.sync.dma_start(
                tv_o.rearrange("(x r) c -> r x c", x=2), cand[:, :, :])
    nc.compile()
    return nc


# ---------------------------------------------------------------- L3 -----
def _build_l3():
    nc = bacc.Bacc("TRN2", target_bir_lowering=False, debug=False)
    hsT = nc.dram_tensor("hsT", (BL, D, S), BF16, kind="ExternalInput").ap()
    R = BL * TOPK  # 512 retrieved rows
    # packed weight inputs (one HWDGE descriptor-gen each instead of six):
    # chain-critical Wq|Wk|seriesT|retrT first, Wv|Wo second
    WX = 2 * D + BL + R
    # fp8e4m3 weights (prescaled by Q8_SCALE): the ACT engine upconverts
    # to bf16 with an exact /64 on device, so the chain matmuls are
    # unchanged and only weight-quantization noise (~4e-3) is added while
    # the weight DMA halves
    wpack_i = nc.dram_tensor("wpack", (D, WX), BF16, kind="ExternalInput").ap()
    wvwo_i = nc.dram_tensor("wvwo", (D, 2 * D), F8E4, kind="ExternalInput").ap()
    # bq|bk host-packed to the device layout (32B descriptors, not 4B)
    bqbk_i = nc.dram_tensor("bqbk", (128, 2 * KJ), F32, kind="ExternalInput").ap()
    sm32_i = nc.dram_tensor("sm32", (BL, D + TOPK + 1), F32,
                            kind="ExternalInput").ap()
    # bv/bo/wgs/wgm/ln_g/ln_b on one partition; a PE ones-matmul
    # replicates them across the 32 batch partitions (a broadcast DMA
    # would move 32x the bytes)
    reps_i = nc.dram_tensor("reps", (6, D), F32, kind="ExternalInput").ap()
    out_o = nc.dram_tensor("out", (BL, D, S), BF16, kind="ExternalOutput").ap()

    with TileContext(nc) as tc:
        with (
            tc.tile_pool(name="wp", bufs=1) as wp,
            tc.tile_pool(name="act", bufs=1) as actp,
            tc.tile_pool(name="sml", bufs=1) as sml,
            tc.tile_pool(name="hidp", bufs=1) as hidp,
            tc.tile_pool(name="psA", bufs=2, space="PSUM") as psA,
        ):
            wpk = wp.tile([128, KJ, WX], BF16, tag="wpack")
            nc.sync.dma_start(
                wpk[:, :, :], wpack_i.rearrange("(j p) x -> p j x", p=128))
            wq = wpk[:, :, 0 * D:1 * D]
            wk = wpk[:, :, 1 * D:2 * D]
            st_t = wpk[:, :, 2 * D:2 * D + BL]
            rt_t = wpk[:, :, 2 * D + BL:]
            wvo8 = wp.tile([128, KJ, 2 * D], F8E4, tag="wvwo8")
            nc.sync.dma_start(
                wvo8[:, :, :], wvwo_i.rearrange("(j p) x -> p j x", p=128))
            wvo = wp.tile([128, KJ, 2 * D], BF16, tag="wvwo")
            nc.scalar.activation(wvo[:, :, :], wvo8[:, :, :], ACT.Copy,
                                 scale=1.0 / Q8_SCALE)
            wv = wvo[:, :, 0:D]
            wo = wvo[:, :, D:2 * D]
            bqbk_t = sml.tile([128, 2, KJ], F32)
            nc.sync.dma_start(
                bqbk_t[:, :, :],
                bqbk_i.rearrange("p (x j) -> p x j", j=KJ))
            bqT = bqbk_t[:, 0, :]
            bkT = bqbk_t[:, 1, :]
            sm32 = sml.tile([BL, D + TOPK + 1], F32)
            nc.sync.dma_start(sm32[:, :], sm32_i[:, :])
            series = sm32[:, 0:D]
            topv = sm32[:, D:D + TOPK]
            bg_t = sm32[:, D + TOPK:D + TOPK + 1]
            rep_t = sml.tile([BL, 6, D], F32)
            nc.sync.dma_start(
                rep_t[:, :, :], reps_i[None, :, :].to_broadcast([BL, 6, D]))
            bv_rep = rep_t[:, 0, :]
            bo_rep = rep_t[:, 1, :]
            wgs_rep = rep_t[:, 2, :]
            wgm_rep = rep_t[:, 3, :]
            lng_rep = rep_t[:, 4, :]
            lnb_rep = rep_t[:, 5, :]
            id32 = sml.tile([32, 32], F32)
            make_identity(nc, id32[:, :])
            eye16 = sml.tile([16, 16], F32)
            make_identity(nc, eye16[:, :])
            ones16 = sml.tile([16, 128], F32)
            nc.vector.memset(ones16[:, :], 1.0)

            # Prefetch the hidden re-reads right behind the small tensors:
            # issued from the sync queue, they stream the 47us of bf16 reads
            # during the attention chain.
            HT_BUFS = 25
            hts = []
            for b in range(BL):
                ht = hidp.tile([128, KJ, S], BF16, tag="hload",
                               bufs=HT_BUFS, name=f"ht{b}")
                nc.sync.dma_start(
                    ht[:, :, :],
                    hsT[b].rearrange("(j p) s -> p j s", p=128))
                hts.append(ht)

            # QpT[e, b] = sum_d WqT[d, e] seriesT[d, b]  (+bq per-partition e)
            qpT = actp.tile([128, KJ, BL], BF16, tag="qpT")
            for eb in range(KJ):
                pq = psA.tile([128, BL], F32, tag="smallmm")
                for dj in range(KJ):
                    nc.tensor.matmul(
                        pq[:, :], wq[:, dj, eb * 128:(eb + 1) * 128],
                        st_t[:, dj, :], start=(dj == 0), stop=(dj == KJ - 1))
                nc.vector.tensor_scalar(
                    qpT[:, eb, :], pq[:, :], bqT[:, eb:eb + 1], None, op0=OP.add)

            # scores[b, k] = SCALE * Qp[:, b].(Kp[:, b*16+k] + bk): per
            # e-block, Kp lands in PSUM, the ACT stage adds bk (per-partition
            # bias) while downcasting to bf16, and one accumulating PE matmul
            # forms the full outer product psc2[b, r] = Qp.T @ (Kp + bk).
            # The block-diagonal entries are then picked out by a SCALE-scaled
            # identity mask + reduce (no cross-partition DMA on this path).
            psc2 = psA.tile([BL, R], F32, tag="psc2")
            for eb in range(KJ):
                pk = psA.tile([128, R], F32, tag="big")
                for dj in range(KJ):
                    nc.tensor.matmul(
                        pk[:, :], wk[:, dj, eb * 128:(eb + 1) * 128],
                        rt_t[:, dj, :], start=(dj == 0), stop=(dj == KJ - 1))
                kp_sb = actp.tile([128, R], BF16, tag="kpsb", bufs=2)
                nc.scalar.activation(
                    kp_sb[:, :], pk[:, :], ACT.Identity, bias=bkT[:, eb:eb + 1])
                nc.tensor.matmul(
                    psc2[:, :], qpT[:, eb, :], kp_sb[:, :],
                    start=(eb == 0), stop=(eb == KJ - 1))
            eyeS = sml.tile([BL, BL], F32)
            make_identity(nc, eyeS[:, :])
            eyeSs = sml.tile([BL, BL], F32)
            nc.vector.tensor_scalar(
                eyeSs[:, :], eyeS[:, :], SCALE, None, op0=OP.mult)
            tmp3 = sml.tile([BL, R], F32)
            nc.vector.tensor_mul(
                tmp3[:, :].rearrange("p (b2 k) -> p b2 k", k=TOPK),
                psc2[:, :].rearrange("p (b2 k) -> p b2 k", k=TOPK),
                eyeSs[:, :, None].to_broadcast([BL, BL, TOPK]))
            scores0 = sml.tile([BL, TOPK], F32)
            nc.vector.tensor_reduce(
                scores0[:, :], tmp3[:, :].rearrange("p (b2 k) -> p k b2", k=TOPK),
                axis=AX.X, op=OP.add)
            pen = sml.tile([BL, TOPK], F32)
            nc.vector.tensor_scalar(
                pen[:, :], topv[:, :], -1.0e30, NEG, op0=OP.is_le, op1=OP.mult)
            mask01 = sml.tile([BL, TOPK], F32)
            nc.vector.tensor_scalar(
                mask01[:, :], topv[:, :], -1.0e30, None, op0=OP.is_gt)
            scores = sml.tile([BL, TOPK], F32)
            nc.vector.tensor_add(scores[:, :], scores0[:, :], pen[:, :])
            nrowmax = sml.tile([BL, 1], F32)
            nc.vector.tensor_reduce(nrowmax[:, :], scores[:, :], axis=AX.X,
                                    op=OP.max, negate=True)
            ex = sml.tile([BL, TOPK], F32)
            nc.scalar.activation(ex[:, :], scores[:, :], ACT.Exp, bias=nrowmax[:, 0:1])
            em = sml.tile([BL, TOPK], F32)
            nc.vector.tensor_mul(em[:, :], ex[:, :], mask01[:, :])
            den = sml.tile([BL, 1], F32)
            nc.vector.tensor_reduce(den[:, :], em[:, :], axis=AX.X, op=OP.add)
            rden = sml.tile([BL, 1], F32)
            nc.vector.reciprocal(rden[:, :], den[:, :])
            attn = sml.tile([BL, TOPK], F32)
            nc.vector.tensor_scalar(
                attn[:, :], em[:, :], rden[:, 0:1], None, op0=OP.mult)

            # mem_out = (sum_k attn_k * retr_k) @ WvT + (sum_k attn_k) * bv.
            # The weighted sum runs in the d-major layout: attn transposed
            # onto 16 partitions, expanded into a k-selective block row,
            # replicated across 128 partitions by a PE ones-matmul, then one
            # DVE mult + k-reduce over rt_t (no 16-step serial accumulate).
            paT = psA.tile([16, BL], F32, tag="smallmm")
            nc.tensor.transpose(paT[:, :], attn[:, :], id32[:, :])
            aT = sml.tile([16, BL], F32)
            nc.scalar.copy(aT[:, :], paT[:, :])
            aTexp = sml.tile([16, R], F32)
            nc.vector.tensor_mul(
                aTexp[:, :].rearrange("c (b k) -> c b k", k=TOPK),
                aT[:, :, None].to_broadcast([16, BL, TOPK]),
                eye16[:, None, :].to_broadcast([16, BL, TOPK]))
            pa = psA.tile([128, R], F32, tag="big")
            nc.tensor.matmul(pa[:, :], ones16[:, :], aTexp[:, :],
                             start=True, stop=True)
            wretTf = actp.tile([128, KJ, BL], F32, tag="wretTf")
            for j in range(KJ):
                prodj = actp.tile([128, R], F32, tag="prodj", bufs=2)
                nc.vector.tensor_mul(prodj[:, :], rt_t[:, j, :], pa[:, :])
                nc.vector.tensor_reduce(
                    wretTf[:, j, :],
                    prodj[:, :].rearrange("p (b k) -> p b k", k=TOPK),
                    axis=AX.X, op=OP.add)
            wretT = actp.tile([128, KJ, BL], BF16, tag="wretT")
            nc.scalar.copy(wretT[:, :, :], wretTf[:, :, :])
            pmv = psA.tile([BL, D], F32, tag="big")
            for j in range(KJ):
                nc.tensor.matmul(
                    pmv[:, :], wretT[:, j, :], wv[:, j, :],
                    start=(j == 0), stop=(j == KJ - 1))
            asum = sml.tile([BL, 1], F32)
            nc.vector.tensor_reduce(asum[:, :], attn[:, :], axis=AX.X, op=OP.add)
            mo = sml.tile([BL, D], F32)
            nc.vector.scalar_tensor_tensor(
                out=mo[:, :], in0=bv_rep[:, :], scalar=asum[:, 0:1],
                in1=pmv[:, :], op0=OP.mult, op1=OP.add)

            # moT via PE transpose, then mo2 = moT.T @ WoT + bo
            moT = actp.tile([128, KJ, BL], BF16, tag="moT")
            for j in range(KJ):
                ptr = psA.tile([128, BL], F32, tag="smallmm")
                nc.tensor.transpose(ptr[:, :], mo[:, j * 128:(j + 1) * 128], id32[:, :])
                nc.scalar.copy(moT[:, j, :], ptr[:, :])
            pmo2 = psA.tile([BL, D], F32, tag="smallmm")
            for j in range(KJ):
                nc.tensor.matmul(
                    pmo2[:, :], moT[:, j, :], wo[:, j, :],
                    start=(j == 0), stop=(j == KJ - 1))
            mo2 = sml.tile([BL, D], F32)
            nc.vector.tensor_add(mo2[:, :], pmo2[:, :], bo_rep[:, :])

            # gate = sigmoid(series.wgs + mo2.wgm + bg); conf = sigmoid(maxsim)
            scr = sml.tile([BL, D], F32, tag="tmpbd", bufs=2)
            a1 = sml.tile([BL, 1], F32)
            nc.vector.scalar_tensor_tensor(
                out=scr[:, :], in0=series[:, :], scalar=1.0, in1=wgs_rep[:, :],
                op0=OP.mult, op1=OP.mult, accum_out=a1[:, :])
            scr2 = sml.tile([BL, D], F32, tag="tmpbd", bufs=2)
            a2 = sml.tile([BL, 1], F32)
            nc.vector.scalar_tensor_tensor(
                out=scr2[:, :], in0=mo2[:, :], scalar=1.0, in1=wgm_rep[:, :],
                op0=OP.mult, op1=OP.mult, accum_out=a2[:, :])
            gsum = sml.tile([BL, 1], F32)
            nc.vector.tensor_add(gsum[:, :], a1[:, :], a2[:, :])
            gsum2 = sml.tile([BL, 1], F32)
            nc.vector.tensor_add(gsum2[:, :], gsum[:, :], bg_t[:, :])
            gate = sml.tile([BL, 1], F32)
            nc.scalar.activation(gate[:, :], gsum2[:, :], ACT.Sigmoid)
            maxsim = sml.tile([BL, 1], F32)
            nc.vector.tensor_reduce(maxsim[:, :], topv[:, :], axis=AX.X, op=OP.max)
            conf = sml.tile([BL, 1], F32)
            nc.scalar.activation(conf[:, :], maxsim[:, :], ACT.Sigmoid)
            gc = sml.tile([BL, 1], F32)
            nc.vector.tensor_mul(gc[:, :], gate[:, :], conf[:, :])
            fused = sml.tile([BL, D], F32)
            nc.vector.scalar_tensor_tensor(
                out=fused[:, :], in0=mo2[:, :], scalar=gc[:, 0:1],
                in1=series[:, :], op0=OP.mult, op1=OP.add)

            # LayerNorm
            fsum = sml.tile([BL, 1], F32)
            nc.vector.tensor_reduce(fsum[:, :], fused[:, :], axis=AX.X, op=OP.add)
            mu = sml.tile([BL, 1], F32)
            nc.vector.tensor_scalar(mu[:, :], fsum[:, :], 1.0 / D, None, op0=OP.mult)
            xc = sml.tile([BL, D], F32)
            nc.vector.tensor_scalar(xc[:, :], fused[:, :], mu[:, 0:1], None, op0=OP.subtract)
            sq = sml.tile([BL, D], F32, tag="tmpbd", bufs=2)
            vs = sml.tile([BL, 1], F32)
            nc.vector.scalar_tensor_tensor(
                out=sq[:, :], in0=xc[:, :], scalar=1.0, in1=xc[:, :],
                op0=OP.mult, op1=OP.mult, accum_out=vs[:, :])
            varp = sml.tile([BL, 1], F32)
            nc.vector.tensor_scalar(
                varp[:, :], vs[:, :], 1.0 / D, LN_EPS, op0=OP.mult, op1=OP.add)
            sd = sml.tile([BL, 1], F32)
            nc.scalar.sqrt(sd[:, :], varp[:, :])
            rsd = sml.tile([BL, 1], F32)
            nc.vector.reciprocal(rsd[:, :], sd[:, :])
            xng = sml.tile([BL, D], F32, tag="tmpbd", bufs=2)
            nc.vector.scalar_tensor_tensor(
                out=xng[:, :], in0=xc[:, :], scalar=rsd[:, 0:1], in1=lng_rep[:, :],
                op0=OP.mult, op1=OP.mult)
            fln = sml.tile([BL, D], F32)
            nc.vector.tensor_add(fln[:, :], xng[:, :], lnb_rep[:, :])
            deltaF = sml.tile([BL, D], F32)
            nc.vector.tensor_sub(deltaF[:, :], fln[:, :], series[:, :])

            # delta onto partitions: deltaT[d, b], f32, via PE transposes
            deltaT = sml.tile([128, KJ, BL], F32)
            for j in range(KJ):
                ptr = psA.tile([128, BL], F32, tag="smallmm")
                nc.tensor.transpose(
                    ptr[:, :], deltaF[:, j * 128:(j + 1) * 128], id32[:, :])
                nc.scalar.copy(deltaT[:, j, :], ptr[:, :])

            # out[b, d, s] = hsT[b, d, s] + deltaT[d, b]: per-partition
            # scalar adds split over DVE/ACT/Pool (all hidden under DMA)
            for b in range(BL):
                ot = hidp.tile([128, KJ, S], BF16, tag="oload", bufs=7,
                               name=f"ot{b}")
                for dj in range(KJ):
                    i = b * KJ + dj
                    r = i % 13
                    if r < 3:
                        nc.vector.tensor_scalar(
                            ot[:, dj, :], hts[b][:, dj, :],
                            deltaT[:, dj, b:b + 1], None, op0=OP.add)
                    elif r < 8:
                        nc.scalar.activation(
                            ot[:, dj, :], hts[b][:, dj, :], ACT.Identity,
                            bias=deltaT[:, dj, b:b + 1])
                    else:
                        nc.gpsimd.tensor_scalar(
                            ot[:, dj, :], hts[b][:, dj, :],
                            deltaT[:, dj, b:b + 1], None, op0=OP.add)
                nc.sync.dma_start(
                    out_o[b].rearrange("(j p) s -> p j s", p=128), ot[:, :, :])
    nc.compile()
    return nc


def _get(name):
    if name not in _programs:
        _programs[name] = {"l1": _build_l1, "l2": _build_l2, "l3": _build_l3}[name]()
    return _programs[name]


def _run(nc, in_maps, tag):
    trace = os.environ.get("KNN_TRACE") == "1"
    res = bass_utils.run_bass_kernel_spmd(
        nc, in_maps, core_ids=list(range(NC)), trace=trace)
    if trace:
        EXEC_NS[tag] = res.exec_time_ns
    return res.results


def kernel(**inputs):
    hs = np.ascontiguousarray(np.asarray(inputs["hidden_states"], np.float32))
    mb = np.ascontiguousarray(np.asarray(inputs["memory_bank"], np.float32))
    Wq, bq = np.asarray(inputs["Wq"], np.float32), np.asarray(inputs["bq"], np.float32)
    Wk, bk = np.asarray(inputs["Wk"], np.float32), np.asarray(inputs["bk"], np.float32)
    Wv, bv = np.asarray(inputs["Wv"], np.float32), np.asarray(inputs["bv"], np.float32)
    Wo, bo = np.asarray(inputs["Wo"], np.float32), np.asarray(inputs["bo"], np.float32)
    Wg, bg = np.asarray(inputs["Wg"], np.float32), np.asarray(inputs["bg"], np.float32)
    ln_g, ln_b = np.asarray(inputs["ln_g"], np.float32), np.asarray(inputs["ln_b"], np.float32)

    # transposed bf16 hidden states, shared by L1 and L3
    hsT = np.ascontiguousarray(hs.astype(NP_BF16).transpose(0, 2, 1))

    # ---- L1: per-batch sums over S, batch-sharded ----
    l1 = _get("l1")
    r1 = _run(l1, [{"hsT": hsT[i * BL:(i + 1) * BL]} for i in range(NC)], "l1")
    sums = np.concatenate([r1[i]["sums"].T for i in range(NC)], axis=0)  # (B, D)
    series = (sums / S).astype(np.float32)
    snorm = np.linalg.norm(series.astype(np.float64), axis=1)
    snorm_safe = np.where(snorm > 0, snorm, 1.0)

    # ---- L2: sims group-max + top-8 groups per shard, bank-sharded ----
    mbT = mb.T  # (D, M) fp32 view
    bankT8 = (mbT * Q8_SCALE).astype(NP_F8E4)  # (D, M) fp8e4m3, prescaled
    qT8 = np.ascontiguousarray((series.T * Q8_SCALE).astype(NP_F8E4))
    l2 = _get("l2")
    pad = np.zeros((D, MLP - ML), NP_F8E4)
    in_maps = [
        {"qT": qT8,
         "bankT": np.ascontiguousarray(
             np.concatenate([bankT8[:, i * ML:(i + 1) * ML], pad], axis=1))}
        for i in range(NC)
    ]
    r2 = _run(l2, in_maps, "l2")
    tv = np.stack([r2[i]["tv"] for i in range(NC)], axis=0)     # (NC, B, 48)
    gvals = tv[:, :, :8].astype(np.float32) / (Q8_SCALE * Q8_SCALE)

    def _u16(lo, hi):
        return (np.ascontiguousarray(tv[:, :, lo:hi]).view(np.uint16)
                .astype(np.int64))

    # unwind the run-8 chain: a position < 8 at any level points into the
    # previous fold's top-8; otherwise it is an offset into that level's
    # region of the group array.  Fold 0 has no predecessor, so its
    # positions are direct group ids.
    nf = len(L2_STAGES)
    posF = _u16(8, 16)
    fold_pos = [_u16(16 + 8 * (nf - 1 - r), 24 + 8 * (nf - 1 - r))
                for r in range(nf)]                             # r = 0..nf-1
    gidx = np.where(posF >= 8, _SG[nf] + posF - 8, -1)
    carry = np.minimum(posF, 7)
    for r in range(nf - 1, 0, -1):
        p = np.take_along_axis(fold_pos[r], carry, axis=2)
        gidx = np.where((gidx < 0) & (p >= 8), _SG[r] + p - 8, gidx)
        carry = np.where(gidx < 0, np.minimum(p, 7), carry)
    p0 = np.take_along_axis(fold_pos[0], carry, axis=2)
    gidx = np.where(gidx < 0, p0, gidx)
    bad = (gidx >= NG) | (gidx < 0)
    gidx = np.where(bad, 0, gidx)

    # candidate columns: 8 groups x 8 cols per (core, query)
    cols = GROUP_COLS[gidx]                                     # (NC,B,8,8)
    valid = (~bad[:, :, :, None]) & (gidx[:, :, :, None] < NG) & (cols < ML)
    grow = cols + (np.arange(NC, dtype=np.int64) * ML)[:, None, None, None]
    grow = np.where(valid, grow, 0)
    rows_q = grow.transpose(1, 0, 2, 3).reshape(B, NC * 64)     # (B, 512)
    valid_q = valid.transpose(1, 0, 2, 3).reshape(B, NC * 64)

    # exact host rescan of the candidate columns (f32)
    sims_sub = np.empty((B, NC * 64), np.float32)
    CH = 32
    for q0 in range(0, B, CH):
        sl = slice(q0, q0 + CH)
        gathered = mb[rows_q[sl]]                               # (CH, 512, D)
        sims_sub[sl] = np.einsum(
            "qkd,qd->qk", gathered, series[sl], optimize=True)
    cosv = sims_sub / snorm_safe[:, None]
    cosv = np.where(valid_q, cosv, -np.inf)
    cosv = np.where(cosv > 0.999, -np.inf, cosv)               # exclude_self
    cosv = np.where(cosv >= THRESH, cosv, -np.inf)             # threshold

    part = np.argpartition(-cosv, TOPK - 1, axis=1)[:, :TOPK]
    topv = np.take_along_axis(cosv, part, axis=1)              # (B, 16)
    topi = np.take_along_axis(rows_q, part, axis=1)            # (B, 16)
    order = np.argsort(-topv, axis=1, kind="stable")
    topv = np.take_along_axis(topv, order, axis=1)
    topi = np.take_along_axis(topi, order, axis=1)

    # Sufficiency: a shard can only hide a true top-16 element if all 8 of
    # its returned group-maxima beat the merged 16th-best value.  On the
    # (never-observed) failure, recompute that query exactly on host.
    v16 = topv[:, TOPK - 1]                                    # (B,)
    g8min = gvals.min(axis=2) / snorm_safe[None, :]            # (NC, B)
    flagged = np.where((g8min > v16[None, :] - SUFF_MARGIN).any(axis=0))[0]
    for q in flagged:
        cos_all = (mb @ series[q]) / snorm_safe[q]
        cos_all = np.where(cos_all > 0.999, -np.inf, cos_all)
        cos_all = np.where(cos_all >= THRESH, cos_all, -np.inf)
        pq = np.argpartition(-cos_all, TOPK - 1)[:TOPK]
        vq = cos_all[pq]
        oq = np.argsort(-vq, kind="stable")
        topv[q] = vq[oq]
        topi[q] = pq[oq]

    if not np.any(topv > -np.inf):
        # nothing retrieved anywhere -> output == hidden_states exactly
        return hs.copy()

    topv_dev = np.where(np.isfinite(topv), topv, NEG).astype(np.float32)
    # guard: gather index for -inf slots is arbitrary but harmless (masked)
    topi = np.where(np.isfinite(topv), topi, 0)

    # ---- L3: attention + gate + LN + broadcast add, batch-sharded ----
    wgs, wgm = np.ascontiguousarray(Wg[0, :D]), np.ascontiguousarray(Wg[0, D:])
    reps = np.ascontiguousarray(np.stack([bv, bo, wgs, wgm, ln_g, ln_b]))
    l3 = _get("l3")
    # fp8 weights, prescaled by Q8_SCALE (device upconverts with /64)
    wcat = np.concatenate([Wq.T, Wk.T], axis=1).astype(NP_BF16)
    wvwo = np.ascontiguousarray(
        (np.concatenate([Wv.T, Wo.T], axis=1) * Q8_SCALE).astype(NP_F8E4))
    bqbk = np.ascontiguousarray(
        np.concatenate([bq, bk]).reshape(2, KJ, 128)
        .transpose(2, 0, 1).reshape(128, 2 * KJ))
    bg_col = np.full((BL, 1), bg[0], np.float32)
    in_maps = []
    for i in range(NC):
        sl = slice(i * BL, (i + 1) * BL)
        idx_flat = topi[sl].reshape(-1)  # (BL*16,)
        wpack = np.concatenate(
            [wcat, series[sl].T.astype(NP_BF16),
             mbT[:, idx_flat].astype(NP_BF16)], axis=1)
        sm32 = np.concatenate(
            [series[sl], topv_dev[sl], bg_col], axis=1).astype(np.float32)
        in_maps.append({
            "hsT": hsT[sl],
            "wpack": np.ascontiguousarray(wpack),
            "wvwo": wvwo,
            "bqbk": bqbk,
            "sm32": np.ascontiguousarray(sm32),
            "reps": reps,
        })
    r3 = _run(l3, in_maps, "l3")
    outT = np.concatenate([r3[i]["out"] for i in range(NC)], axis=0)  # (B,D,S)
    return outT.transpose(0, 2, 1).astype(np.float32)


# revision 64
# speedup vs baseline: 1.0054x; 1.0054x over previous
"""Memory-augmented forecaster kernel for 8 Trainium2 NeuronCores.

Pipeline (3 SPMD launches; host does only sharding/layout/merge between):
  All hidden-state traffic uses a host-transposed layout hsT[b] = (D, S):
  the per-batch delta broadcast becomes a per-partition-scalar add and the
  S-mean becomes a free-axis reduction, so both split across the DVE, ACT
  and Pool engines with no PE broadcast matmuls and no staging.

  L1 (batch-sharded, 32 queries/core, ~55us): sums[b, :] = sum_S hsT[b]
      via free-axis reductions interleaved over DVE/ACT/Pool (Pool adds
      the two S-halves, DVE finishes), hidden under the bf16 hidden-state
      read (DMA-bound, 47us floor).  The last two batches are split
      per-dj across engines to shorten the tail.  Host divides by S.
  L2 (bank-sharded, 12500 rows/core padded to 12504, ~37us): sims =
      q @ bankT as an fp8e4m3 DoubleRow PE matmul (256-deep contraction,
      0.5 cyc/col, fp32 PSUM; inputs prescaled by 64 so the fp8 grid
      centers).  Per supertile (up to 1024 cols of DMA tiles in one 4-bank
      PSUM buffer) the ACT engine stages PSUM to f16 once and the DVE computes
      per-8-col group maxima as three packed halving rounds (2x mode;
      group g = stride-sw/8 column set).  A chained running-top-8
      (Max8/MaxIndex8 after supertiles 3/6/9/10) folds finished regions
      under the sweep so the final scan stays short; the host unwinds the
      position chain.  The device output is selection-only: the host
      rescans the 8 cores x 8 groups
      x 8 cols = 512 candidate columns per query exactly in f32 (so fp8
      noise, ~1.3e-3 on sims, never reaches the result), with a
      per-query exact-recompute fallback if the provable sufficiency
      bound fails (never fires on random data).
  L3 (batch-sharded, ~104us): gated cross-attention over the top-16
      memories (weighted-sum pushed before the Wv projection), gating,
      LayerNorm; delta = LN(fused) - series is PE-transposed onto
      partitions and added to hsT as a per-partition scalar on
      DVE/ACT/Pool.  Wv|Wo ship fp8e4m3 (prescaled by 64; ACT
      upconverts to bf16 on device, adding only benign weight
      quantization noise -- Wq/Wk stay bf16 since score noise flips
      attention near-ties) and bq|bk is host-packed to the device layout.
      Output is written bf16 (halves the write traffic; rel-err ~5.4e-3
      vs the 2e-2 budget) and the host transposes/upcasts back to
      (B, S, D) f32.  DMA-bound at ~100us busy of its traffic floor.
"""

import os
import numpy as np

import concourse.bacc as bacc
import concourse.mybir as mybir
from concourse import bass_utils
from concourse.tile import TileContext
from concourse.masks import make_identity

F32 = mybir.dt.float32
F16 = mybir.dt.float16
BF16 = mybir.dt.bfloat16
F8E4 = mybir.dt.float8e4
U16 = mybir.dt.uint16
AX = mybir.AxisListType
OP = mybir.AluOpType
ACT = mybir.ActivationFunctionType

NP_BF16 = mybir.dt.np(BF16)
NP_F8E4 = mybir.dt.np(F8E4)
Q8_SCALE = 64.0       # power-of-2 prescale centering fp8e4m3's range

B, S, D = 256, 512, 512
M, TOPK = 100000, 16
NC = 8
BL = B // NC          # 32 queries per core (L1/L3)
ML = M // NC          # 12500 bank rows per core (L2)
MLP = 12504           # padded to a multiple of 8 (4 zero columns)
G = 8                 # L2 group width for the PSUM group-max
NG = MLP // G         # 1563 groups per shard
CT = 512              # L2 DMA column tile
# L2 supertiles: column tiles sharing one PSUM buffer so the ACT
# f16-staging and DVE halving rounds run as wide ops.  The first supertile
# is a single narrow tile (earliest possible first stage) and the last is
# the 216-col remainder (shortest possible tail chain).
L2_TILES = [128, 384] + [CT] * 23 + [216]
assert sum(L2_TILES) == MLP and all(w % G == 0 for w in L2_TILES)
L2_C0 = [sum(L2_TILES[:i]) for i in range(len(L2_TILES))]
L2_ST = [[0], [1, 2]] + [[3 + 2 * k, 4 + 2 * k] for k in range(11)] + [[25]]
L2_SUPER = [sum(L2_TILES[t] for t in ts) for ts in L2_ST]
assert sum(L2_SUPER) == MLP
KJ = D // 128         # 4 contraction subtiles
# L2 chained running-top-8: after each listed supertile a Max8/MaxIndex8
# folds the finished region (plus the previous run-8) into a new run-8, so
# every fold is short and hides in the sweep; the final scan after the
# last supertile covers only [run8 | 27 tail groups].  gmx regions are
# separated by 8-slot run buffers so every scan is contiguous.
L2_STAGES = [3, 6, 9, 10]               # supertile index each fold follows
_SG = [0, 384, 768, 1152, 1280, NG]     # group-range region boundaries
GPAD = 5                                # pad groups after the tail region


def _group_cols_lut():
    """Padded-shard columns of each group id: group g of an sw-wide
    supertile at s0 covers {s0 + (g - s0/8) + (sw/8)*i} (the device's
    halving rounds pair stride-sw/8 columns)."""
    lut = np.zeros((NG, G), np.int64)
    s0 = 0
    for sw in L2_SUPER:
        g0, ng = s0 // G, sw // G
        loc = np.arange(ng)
        lut[g0:g0 + ng] = s0 + loc[:, None] + ng * np.arange(G)[None, :]
        s0 += sw
    return lut


GROUP_COLS = _group_cols_lut()
SCALE = D ** -0.5
LN_EPS = 1e-5
GATE_TEMP = 1.0
THRESH = 0.0
NEG = -1.0e38
SUFF_MARGIN = 1.2e-2  # device fp8/f16 vs host-f32 sim slack (~9 sigma)

EXEC_NS = {}

_programs = {}


# ---------------------------------------------------------------- L1 -----
def _build_l1():
    nc = bacc.Bacc("TRN2", target_bir_lowering=False, debug=False)
    hsT = nc.dram_tensor("hsT", (BL, D, S), BF16, kind="ExternalInput").ap()
    sums_o = nc.dram_tensor("sums", (D, BL), F32, kind="ExternalOutput").ap()

    # Spread the per-batch S-reduction across DVE/ACT/Pool so every engine
    # stays under the DMA stream (~47us).  gpsimd tensor_reduce only does
    # partition-axis reductions, so Pool instead adds the two S-halves
    # and DVE finishes the half-width reduce.  Engines must be interleaved
    # in program order (per-engine queues execute in order); the final two
    # batches are split per-dj across engines so the tail after the last
    # DMA is ~1.3us instead of a full batch reduction.
    sched = list("vapvpavp" * 4)
    sched[30] = sched[31] = "s"

    with TileContext(nc) as tc:
        with (
            tc.tile_pool(name="hidp", bufs=20) as hidp,
            tc.tile_pool(name="sml", bufs=1) as sml,
            tc.tile_pool(name="scr", bufs=3) as scrp,
        ):
            sm = sml.tile([128, KJ, BL], F32)
            for b in range(BL):
                t = hidp.tile([128, KJ, S], BF16, tag="hload")
                nc.sync.dma_start(
                    t[:, :, :], hsT[b].rearrange("(j p) s -> p j s", p=128))
                eng = sched[b]
                if eng == "v":
                    nc.vector.tensor_reduce(
                        sm[:, :, b], t[:, :, :], axis=AX.X, op=OP.add)
                elif eng == "p":
                    # f32 halves tile: exact, and keeps DVE's share small
                    half = scrp.tile([128, KJ, S // 2], F32, tag="phalf")
                    nc.gpsimd.tensor_add(
                        half[:, :, :], t[:, :, :S // 2], t[:, :, S // 2:])
                    nc.vector.tensor_reduce(
                        sm[:, :, b], half[:, :, :], axis=AX.X, op=OP.add)
                elif eng == "a":
                    for dj in range(KJ):
                        scr = scrp.tile([128, S], F32, tag="ascr")
                        nc.scalar.activation(
                            scr[:, :], t[:, dj, :], ACT.Copy,
                            accum_out=sm[:, dj, b:b + 1])
                else:  # split: one dj per engine, all in parallel
                    nc.vector.tensor_reduce(
                        sm[:, 0:1, b], t[:, 0:1, :], axis=AX.X, op=OP.add)
                    scr = scrp.tile([128, S], F32, tag="ascr")
                    nc.scalar.activation(
                        scr[:, :], t[:, 1, :], ACT.Copy,
                        accum_out=sm[:, 1, b:b + 1])
                    half = scrp.tile([128, 1, S // 2], F32, tag="phalf")
                    nc.gpsimd.tensor_add(
                        half[:, :, :], t[:, 2:3, :S // 2], t[:, 2:3, S // 2:])
                    nc.vector.tensor_reduce(
                        sm[:, 2:3, b], half[:, :, :], axis=AX.X, op=OP.add)
                    nc.vector.tensor_reduce(
                        sm[:, 3:4, b], t[:, 3:4, :], axis=AX.X, op=OP.add)
            nc.sync.dma_start(
                sums_o.rearrange("(j p) b -> p j b", p=128), sm[:, :, :])
    nc.compile()
    return nc


# ---------------------------------------------------------------- L2 -----
def _build_l2():
    nc = bacc.Bacc("TRN2", target_bir_lowering=False, debug=False)
    # fp8e4m3 inputs (prescaled by Q8_SCALE on host): the PE runs DoubleRow
    # (256-deep contraction, 0.5 cyc/col) and the bank read halves.  Device
    # sims are selection-only -- the host rescans winning groups in f32 --
    # so the ~1.3e-3 fp8 sim noise only widens the sufficiency margin.
    qT = nc.dram_tensor("qT", (D, B), F8E4, kind="ExternalInput").ap()
    bankT = nc.dram_tensor("bankT", (D, MLP), F8E4, kind="ExternalInput").ap()
    # per query: 8 group-max values (f16) then 4 position octets (u16):
    # final, then stage2/1/0 -- host unwinds the run-8 chain
    tv_o = nc.dram_tensor("tv", (B, 48), F16, kind="ExternalOutput").ap()

    with TileContext(nc) as tc:
        with (
            tc.tile_pool(name="qp", bufs=1) as qp,
            tc.tile_pool(name="bkp", bufs=10) as bkp,
            tc.tile_pool(name="stg", bufs=3) as stg,
            tc.tile_pool(name="outp", bufs=1) as outp,
            tc.tile_pool(name="ps", bufs=2, space="PSUM") as psp,
        ):
            qt = qp.tile([128, 2, 2, B], F8E4)
            nc.sync.dma_start(
                qt[:, :, :, :],
                qT.rearrange("(j i p) b -> p j i b", p=128, i=2))
            # group maxima with an 8-slot run buffer after each region
            gx_w = NG + GPAD + 8 * len(L2_STAGES)
            gmx = outp.tile([128, 2, gx_w], F16)
            nc.vector.memset(gmx[:, :, gx_w - GPAD:], -60000.0)
            cand = outp.tile([128, 2, 48], F16)
            # gmx offset of group g (region r gets r*8 extra) and the
            # scan window [lo, hi) -> run-slot of each chain stage
            goff = lambda g: g + 8 * sum(1 for b in _SG[1:-1] if g >= b)
            stage_win = []
            lo = 0
            nwin = len(L2_STAGES) + 1
            for r in range(nwin):
                hi = goff(_SG[r + 1] - 1) + 1 if r < nwin - 1 else gx_w
                stage_win.append((lo, hi))
                lo = hi
            assert all(b - a <= 16384 for a, b in stage_win)
            bk_re = bankT.rearrange("(j i p) c -> p j i c", p=128, i=2)
            for sidx, sw in enumerate(L2_SUPER):
                s0 = sum(L2_SUPER[:sidx])
                g0 = s0 // G
                # per-blk stride padded to 1024 f32 so every matmul output
                # stays inside whole PSUM banks
                pt = psp.tile([128, 2, 1024], F32, tag="ps")
                off = 0
                for t in L2_ST[sidx]:
                    cw, c0 = L2_TILES[t], L2_C0[t]
                    bk = bkp.tile([128, 2, 2, CT], F8E4, tag="bk")
                    nc.sync.dma_start(
                        bk[:, :, :, :cw], bk_re[:, :, :, c0:c0 + cw])
                    for blk in range(2):
                        for j in range(2):
                            nc.tensor.matmul(
                                pt[:, blk, off:off + cw],
                                qt[:, j, :, blk * 128:(blk + 1) * 128],
                                bk[:, j, :, :cw],
                                start=(j == 0), stop=(j == 1),
                                perf_mode=mybir.MatmulPerfMode.DoubleRow,
                            )
                    off += cw
                # grouped max via one f16 staging op + 3 packed halving
                # rounds (DVE 2x mode; TensorReduce gets no 2x).  Group g of
                # this supertile covers columns {s0 + g + (sw/8)*i}.  The
                # last supertile runs per query block so block 0's final
                # top-8 overlaps block 1's staging.
                d0 = goff(g0)
                last = sidx == len(L2_SUPER) - 1
                for blk in ([0, 1] if last else [slice(None)]):
                    st = stg.tile([128, 2, 1024], F16, tag="st")
                    nc.scalar.copy(st[:, blk, :sw], pt[:, blk, :sw])
                    r1 = stg.tile([128, 2, 512], F16, tag="r1")
                    nc.vector.tensor_tensor(
                        r1[:, blk, :sw // 2], st[:, blk, :sw // 2],
                        st[:, blk, sw // 2:sw], op=OP.max)
                    r2 = stg.tile([128, 2, 256], F16, tag="r2")
                    nc.vector.tensor_tensor(
                        r2[:, blk, :sw // 4], r1[:, blk, :sw // 4],
                        r1[:, blk, sw // 4:sw // 2], op=OP.max)
                    nc.vector.tensor_tensor(
                        gmx[:, blk, d0:d0 + sw // G], r2[:, blk, :sw // 8],
                        r2[:, blk, sw // 8:sw // 4], op=OP.max)
                    if last:
                        lo, hi = stage_win[-1]
                        nc.vector.max(cand[:, blk, 0:8], gmx[:, blk, lo:hi])
                        nc.vector.max_index(
                            cand[:, blk, 8:16].bitcast(U16),
                            cand[:, blk, 0:8], gmx[:, blk, lo:hi])
                if sidx in L2_STAGES:
                    # fold the finished region (+ previous run-8) into a new
                    # run-8; each short scan hides under the remaining sweep
                    r = L2_STAGES.index(sidx)
                    lo, hi = stage_win[r]
                    nf = len(L2_STAGES)
                    for blk in range(2):
                        nc.vector.max(gmx[:, blk, hi:hi + 8],
                                      gmx[:, blk, lo:hi])
                        nc.vector.max_index(
                            cand[:, blk, 16 + 8 * (nf - 1 - r):
                                 24 + 8 * (nf - 1 - r)].bitcast(U16),
                            gmx[:, blk, hi:hi + 8], gmx[:, blk, lo:hi])
            nc.sync.dma_start(
                tv_o.rearrange("(x r) c -> r x c", x=2), cand[:, :, :])
    nc.compile()
    return nc


# ---------------------------------------------------------------- L3 -----
def _build_l3():
    nc = bacc.Bacc("TRN2", target_bir_lowering=False, debug=False)
    hsT = nc.dram_tensor("hsT", (BL, D, S), BF16, kind="ExternalInput").ap()
    R = BL * TOPK  # 512 retrieved rows
    # packed weight inputs (one HWDGE descriptor-gen each instead of six):
    # chain-critical Wq|Wk|seriesT|retrT first, Wv|Wo second
    WX = 2 * D + BL + R
    # fp8e4m3 weights (prescaled by Q8_SCALE): the ACT engine upconverts
    # to bf16 with an exact /64 on device, so the chain matmuls are
    # unchanged and only weight-quantization noise (~4e-3) is added while
    # the weight DMA halves
    wpack_i = nc.dram_tensor("wpack", (D, WX), BF16, kind="ExternalInput").ap()
    wvwo_i = nc.dram_tensor("wvwo", (D, 2 * D), F8E4, kind="ExternalInput").ap()
    # bq|bk host-packed to the device layout (32B descriptors, not 4B)
    bqbk_i = nc.dram_tensor("bqbk", (128, 2 * KJ), F32, kind="ExternalInput").ap()
    sm32_i = nc.dram_tensor("sm32", (BL, D + TOPK + 1), F32,
                            kind="ExternalInput").ap()
    # bv/bo/wgs/wgm/ln_g/ln_b on one partition; a PE ones-matmul
    # replicates them across the 32 batch partitions (a broadcast DMA
    # would move 32x the bytes)
    reps_i = nc.dram_tensor("reps", (6, D), F32, kind="ExternalInput").ap()
    out_o = nc.dram_tensor("out", (BL, D, S), BF16, kind="ExternalOutput").ap()

    with TileContext(nc) as tc:
        with (
            tc.tile_pool(name="wp", bufs=1) as wp,
            tc.tile_pool(name="act", bufs=1) as actp,
            tc.tile_pool(name="sml", bufs=1) as sml,
            tc.tile_pool(name="hidp", bufs=1) as hidp,
            tc.tile_pool(name="psA", bufs=2, space="PSUM") as psA,
        ):
            wpk = wp.tile([128, KJ, WX], BF16, tag="wpack")
            nc.sync.dma_start(
                wpk[:, :, :], wpack_i.rearrange("(j p) x -> p j x", p=128))
            wq = wpk[:, :, 0 * D:1 * D]
            wk = wpk[:, :, 1 * D:2 * D]
            st_t = wpk[:, :, 2 * D:2 * D + BL]
            rt_t = wpk[:, :, 2 * D + BL:]
            wvo8 = wp.tile([128, KJ, 2 * D], F8E4, tag="wvwo8")
            nc.sync.dma_start(
                wvo8[:, :, :], wvwo_i.rearrange("(j p) x -> p j x", p=128))
            wvo = wp.tile([128, KJ, 2 * D], BF16, tag="wvwo")
            nc.scalar.activation(wvo[:, :, :], wvo8[:, :, :], ACT.Copy,
                                 scale=1.0 / Q8_SCALE)
            wv = wvo[:, :, 0:D]
            wo = wvo[:, :, D:2 * D]
            bqbk_t = sml.tile([128, 2, KJ], F32)
            nc.sync.dma_start(
                bqbk_t[:, :, :],
                bqbk_i.rearrange("p (x j) -> p x j", j=KJ))
            bqT = bqbk_t[:, 0, :]
            bkT = bqbk_t[:, 1, :]
            sm32 = sml.tile([BL, D + TOPK + 1], F32)
            nc.sync.dma_start(sm32[:, :], sm32_i[:, :])
            series = sm32[:, 0:D]
            topv = sm32[:, D:D + TOPK]
            bg_t = sm32[:, D + TOPK:D + TOPK + 1]
            rep_t = sml.tile([BL, 6, D], F32)
            nc.sync.dma_start(
                rep_t[:, :, :], reps_i[None, :, :].to_broadcast([BL, 6, D]))
            bv_rep = rep_t[:, 0, :]
            bo_rep = rep_t[:, 1, :]
            wgs_rep = rep_t[:, 2, :]
            wgm_rep = rep_t[:, 3, :]
            lng_rep = rep_t[:, 4, :]
            lnb_rep = rep_t[:, 5, :]
            id32 = sml.tile([32, 32], F32)
            make_identity(nc, id32[:, :])
            eye16 = sml.tile([16, 16], F32)
            make_identity(nc, eye16[:, :])
            ones16 = sml.tile([16, 128], F32)
            nc.vector.memset(ones16[:, :], 1.0)

            # Prefetch the hidden re-reads right behind the small tensors:
            # issued from the sync queue, they stream the 47us of bf16 reads
            # during the attention chain.
            HT_BUFS = 25
            hts = []
            for b in range(BL):
                ht = hidp.tile([128, KJ, S], BF16, tag="hload",
                               bufs=HT_BUFS, name=f"ht{b}")
                nc.sync.dma_start(
                    ht[:, :, :],
                    hsT[b].rearrange("(j p) s -> p j s", p=128))
                hts.append(ht)

            # QpT[e, b] = sum_d WqT[d, e] seriesT[d, b]  (+bq per-partition e)
            qpT = actp.tile([128, KJ, BL], BF16, tag="qpT")
            for eb in range(KJ):
                pq = psA.tile([128, BL], F32, tag="smallmm")
                for dj in range(KJ):
                    nc.tensor.matmul(
                        pq[:, :], wq[:, dj, eb * 128:(eb + 1) * 128],
                        st_t[:, dj, :], start=(dj == 0), stop=(dj == KJ - 1))
                nc.vector.tensor_scalar(
                    qpT[:, eb, :], pq[:, :], bqT[:, eb:eb + 1], None, op0=OP.add)

            # scores[b, k] = SCALE * Qp[:, b].(Kp[:, b*16+k] + bk): per
            # e-block, Kp lands in PSUM, the ACT stage adds bk (per-partition
            # bias) while downcasting to bf16, and one accumulating PE matmul
            # forms the full outer product psc2[b, r] = Qp.T @ (Kp + bk).
            # The block-diagonal entries are then picked out by a SCALE-scaled
            # identity mask + reduce (no cross-partition DMA on this path).
            psc2 = psA.tile([BL, R], F32, tag="psc2")
            for eb in range(KJ):
                pk = psA.tile([128, R], F32, tag="big")
                for dj in range(KJ):
                    nc.tensor.matmul(
                        pk[:, :], wk[:, dj, eb * 128:(eb + 1) * 128],
                        rt_t[:, dj, :], start=(dj == 0), stop=(dj == KJ - 1))
                kp_sb = actp.tile([128, R], BF16, tag="kpsb", bufs=2)
                nc.scalar.activation(
                    kp_sb[:, :], pk[:, :], ACT.Identity, bias=bkT[:, eb:eb + 1])
                nc.tensor.matmul(
                    psc2[:, :], qpT[:, eb, :], kp_sb[:, :],
                    start=(eb == 0), stop=(eb == KJ - 1))
            eyeS = sml.tile([BL, BL], F32)
            make_identity(nc, eyeS[:, :])
            eyeSs = sml.tile([BL, BL], F32)
            nc.vector.tensor_scalar(
                eyeSs[:, :], eyeS[:, :], SCALE, None, op0=OP.mult)
            tmp3 = sml.tile([BL, R], F32)
            nc.vector.tensor_mul(
                tmp3[:, :].rearrange("p (b2 k) -> p b2 k", k=TOPK),
                psc2[:, :].rearrange("p (b2 k) -> p b2 k", k=TOPK),
                eyeSs[:, :, None].to_broadcast([BL, BL, TOPK]))
            scores0 = sml.tile([BL, TOPK], F32)
            nc.vector.tensor_reduce(
                scores0[:, :], tmp3[:, :].rearrange("p (b2 k) -> p k b2", k=TOPK),
                axis=AX.X, op=OP.add)
            pen = sml.tile([BL, TOPK], F32)
            nc.vector.tensor_scalar(
                pen[:, :], topv[:, :], -1.0e30, NEG, op0=OP.is_le, op1=OP.mult)
            mask01 = sml.tile([BL, TOPK], F32)
            nc.vector.tensor_scalar(
                mask01[:, :], topv[:, :], -1.0e30, None, op0=OP.is_gt)
            scores = sml.tile([BL, TOPK], F32)
            nc.vector.tensor_add(scores[:, :], scores0[:, :], pen[:, :])
            nrowmax = sml.tile([BL, 1], F32)
            nc.vector.tensor_reduce(nrowmax[:, :], scores[:, :], axis=AX.X,
                                    op=OP.max, negate=True)
            ex = sml.tile([BL, TOPK], F32)
            nc.scalar.activation(ex[:, :], scores[:, :], ACT.Exp, bias=nrowmax[:, 0:1])
            em = sml.tile([BL, TOPK], F32)
            nc.vector.tensor_mul(em[:, :], ex[:, :], mask01[:, :])
            den = sml.tile([BL, 1], F32)
            nc.vector.tensor_reduce(den[:, :], em[:, :], axis=AX.X, op=OP.add)
            rden = sml.tile([BL, 1], F32)
            nc.vector.reciprocal(rden[:, :], den[:, :])
            attn = sml.tile([BL, TOPK], F32)
            nc.vector.tensor_scalar(
                attn[:, :], em[:, :], rden[:, 0:1], None, op0=OP.mult)

            # mem_out = (sum_k attn_k * retr_k) @ WvT + (sum_k attn_k) * bv.
            # The weighted sum runs in the d-major layout: attn transposed
            # onto 16 partitions, expanded into a k-selective block row,
            # replicated across 128 partitions by a PE ones-matmul, then one
            # DVE mult + k-reduce over rt_t (no 16-step serial accumulate).
            paT = psA.tile([16, BL], F32, tag="smallmm")
            nc.tensor.transpose(paT[:, :], attn[:, :], id32[:, :])
            aT = sml.tile([16, BL], F32)
            nc.scalar.copy(aT[:, :], paT[:, :])
            aTexp = sml.tile([16, R], F32)
            nc.vector.tensor_mul(
                aTexp[:, :].rearrange("c (b k) -> c b k", k=TOPK),
                aT[:, :, None].to_broadcast([16, BL, TOPK]),
                eye16[:, None, :].to_broadcast([16, BL, TOPK]))
            pa = psA.tile([128, R], F32, tag="big")
            nc.tensor.matmul(pa[:, :], ones16[:, :], aTexp[:, :],
                             start=True, stop=True)
            wretTf = actp.tile([128, KJ, BL], F32, tag="wretTf")
            for j in range(KJ):
                prodj = actp.tile([128, R], F32, tag="prodj", bufs=2)
                nc.vector.tensor_mul(prodj[:, :], rt_t[:, j, :], pa[:, :])
                nc.vector.tensor_reduce(
                    wretTf[:, j, :],
                    prodj[:, :].rearrange("p (b k) -> p b k", k=TOPK),
                    axis=AX.X, op=OP.add)
            wretT = actp.tile([128, KJ, BL], BF16, tag="wretT")
            nc.scalar.copy(wretT[:, :, :], wretTf[:, :, :])
            pmv = psA.tile([BL, D], F32, tag="big")
            for j in range(KJ):
                nc.tensor.matmul(
                    pmv[:, :], wretT[:, j, :], wv[:, j, :],
                    start=(j == 0), stop=(j == KJ - 1))
            asum = sml.tile([BL, 1], F32)
            nc.vector.tensor_reduce(asum[:, :], attn[:, :], axis=AX.X, op=OP.add)
            mo = sml.tile([BL, D], F32)
            nc.vector.scalar_tensor_tensor(
                out=mo[:, :], in0=bv_rep[:, :], scalar=asum[:, 0:1],
                in1=pmv[:, :], op0=OP.mult, op1=OP.add)

            # moT via PE transpose, then mo2 = moT.T @ WoT + bo
            moT = actp.tile([128, KJ, BL], BF16, tag="moT")
            for j in range(KJ):
                ptr = psA.tile([128, BL], F32, tag="smallmm")
                nc.tensor.transpose(ptr[:, :], mo[:, j * 128:(j + 1) * 128], id32[:, :])
                nc.scalar.copy(moT[:, j, :], ptr[:, :])
            pmo2 = psA.tile([BL, D], F32, tag="smallmm")
            for j in range(KJ):
                nc.tensor.matmul(
                    pmo2[:, :], moT[:, j, :], wo[:, j, :],
                    start=(j == 0), stop=(j == KJ - 1))
            mo2 = sml.tile([BL, D], F32)
            nc.vector.tensor_add(mo2[:, :], pmo2[:, :], bo_rep[:, :])

            # gate = sigmoid(series.wgs + mo2.wgm + bg); conf = sigmoid(maxsim)
            scr = sml.tile([BL, D], F32, tag="tmpbd", bufs=2)
            a1 = sml.tile([BL, 1], F32)
            nc.vector.scalar_tensor_tensor(
                out=scr[:, :], in0=series[:, :], scalar=1.0, in1=wgs_rep[:, :],
                op0=OP.mult, op1=OP.mult, accum_out=a1[:, :])
            scr2 = sml.tile([BL, D], F32, tag="tmpbd", bufs=2)
            a2 = sml.tile([BL, 1], F32)
            nc.vector.scalar_tensor_tensor(
                out=scr2[:, :], in0=mo2[:, :], scalar=1.0, in1=wgm_rep[:, :],
                op0=OP.mult, op1=OP.mult, accum_out=a2[:, :])
            gsum = sml.tile([BL, 1], F32)
            nc.vector.tensor_add(gsum[:, :], a1[:, :], a2[:, :])
            gsum2 = sml.tile([BL, 1], F32)
            nc.vector.tensor_add(gsum2[:, :], gsum[:, :], bg_t[:, :])
            gate = sml.tile([BL, 1], F32)
            nc.scalar.activation(gate[:, :], gsum2[:, :], ACT.Sigmoid)
            maxsim = sml.tile([BL, 1], F32)
            nc.vector.tensor_reduce(maxsim[:, :], topv[:, :], axis=AX.X, op=OP.max)
            conf = sml.tile([BL, 1], F32)
            nc.scalar.activation(conf[:, :], maxsim[:, :], ACT.Sigmoid)
            gc = sml.tile([BL, 1], F32)
            nc.vector.tensor_mul(gc[:, :], gate[:, :], conf[:, :])
            fused = sml.tile([BL, D], F32)
            nc.vector.scalar_tensor_tensor(
                out=fused[:, :], in0=mo2[:, :], scalar=gc[:, 0:1],
                in1=series[:, :], op0=OP.mult, op1=OP.add)

            # LayerNorm
            fsum = sml.tile([BL, 1], F32)
            nc.vector.tensor_reduce(fsum[:, :], fused[:, :], axis=AX.X, op=OP.add)
            mu = sml.tile([BL, 1], F32)
            nc.vector.tensor_scalar(mu[:, :], fsum[:, :], 1.0 / D, None, op0=OP.mult)
            xc = sml.tile([BL, D], F32)
            nc.vector.tensor_scalar(xc[:, :], fused[:, :], mu[:, 0:1], None, op0=OP.subtract)
            sq = sml.tile([BL, D], F32, tag="tmpbd", bufs=2)
            vs = sml.tile([BL, 1], F32)
            nc.vector.scalar_tensor_tensor(
                out=sq[:, :], in0=xc[:, :], scalar=1.0, in1=xc[:, :],
                op0=OP.mult, op1=OP.mult, accum_out=vs[:, :])
            varp = sml.tile([BL, 1], F32)
            nc.vector.tensor_scalar(
                varp[:, :], vs[:, :], 1.0 / D, LN_EPS, op0=OP.mult, op1=OP.add)
            sd = sml.tile([BL, 1], F32)
            nc.scalar.sqrt(sd[:, :], varp[:, :])
            rsd = sml.tile([BL, 1], F32)
            nc.vector.reciprocal(rsd[:, :], sd[:, :])
            xng = sml.tile([BL, D], F32, tag="tmpbd", bufs=2)
            nc.vector.scalar_tensor_tensor(
                out=xng[:, :], in0=xc[:, :], scalar=rsd[:, 0:1], in1=lng_rep[:, :],
                op0=OP.mult, op1=OP.mult)
            fln = sml.tile([BL, D], F32)
            nc.vector.tensor_add(fln[:, :], xng[:, :], lnb_rep[:, :])
            deltaF = sml.tile([BL, D], F32)
            nc.vector.tensor_sub(deltaF[:, :], fln[:, :], series[:, :])

            # delta onto partitions: deltaT[d, b], f32, via PE transposes
            deltaT = sml.tile([128, KJ, BL], F32)
            for j in range(KJ):
                ptr = psA.tile([128, BL], F32, tag="smallmm")
                nc.tensor.transpose(
                    ptr[:, :], deltaF[:, j * 128:(j + 1) * 128], id32[:, :])
                nc.scalar.copy(deltaT[:, j, :], ptr[:, :])

            # out[b, d, s] = hsT[b, d, s] + deltaT[d, b]: per-partition
            # scalar adds split over DVE/ACT/Pool (all hidden under DMA)
            for b in range(BL):
                ot = hidp.tile([128, KJ, S], BF16, tag="oload", bufs=7,
                               name=f"ot{b}")
                for dj in range(KJ):
                    i = b * KJ + dj
                    r = i % 13
                    if r < 3:
                        nc.vector.tensor_scalar(
                            ot[:, dj, :], hts[b][:, dj, :],
                            deltaT[:, dj, b:b + 1], None, op0=OP.add)
                    elif r < 8:
                        nc.scalar.activation(
                            ot[:, dj, :], hts[b][:, dj, :], ACT.Identity,
                            bias=deltaT[:, dj, b:b + 1])
                    else:
                        nc.gpsimd.tensor_scalar(
                            ot[:, dj, :], hts[b][:, dj, :],
                            deltaT[:, dj, b:b + 1], None, op0=OP.add)
                nc.sync.dma_start(
                    out_o[b].rearrange("(j p) s -> p j s", p=128), ot[:, :, :])
    nc.compile()
    return nc


def _get(name):
    if name not in _programs:
        _programs[name] = {"l1": _build_l1, "l2": _build_l2, "l3": _build_l3}[name]()
    return _programs[name]


def _run(nc, in_maps, tag):
    trace = os.environ.get("KNN_TRACE") == "1"
    res = bass_utils.run_bass_kernel_spmd(
        nc, in_maps, core_ids=list(range(NC)), trace=trace)
    if trace:
        EXEC_NS[tag] = res.exec_time_ns
    return res.results


def kernel(**inputs):
    hs = np.ascontiguousarray(np.asarray(inputs["hidden_states"], np.float32))
    mb = np.ascontiguousarray(np.asarray(inputs["memory_bank"], np.float32))
    Wq, bq = np.asarray(inputs["Wq"], np.float32), np.asarray(inputs["bq"], np.float32)
    Wk, bk = np.asarray(inputs["Wk"], np.float32), np.asarray(inputs["bk"], np.float32)
    Wv, bv = np.asarray(inputs["Wv"], np.float32), np.asarray(inputs["bv"], np.float32)
    Wo, bo = np.asarray(inputs["Wo"], np.float32), np.asarray(inputs["bo"], np.float32)
    Wg, bg = np.asarray(inputs["Wg"], np.float32), np.asarray(inputs["bg"], np.float32)
    ln_g, ln_b = np.asarray(inputs["ln_g"], np.float32), np.asarray(inputs["ln_b"], np.float32)

    # transposed bf16 hidden states, shared by L1 and L3
    hsT = np.ascontiguousarray(hs.astype(NP_BF16).transpose(0, 2, 1))

    # ---- L1: per-batch sums over S, batch-sharded ----
    l1 = _get("l1")
    r1 = _run(l1, [{"hsT": hsT[i * BL:(i + 1) * BL]} for i in range(NC)], "l1")
    sums = np.concatenate([r1[i]["sums"].T for i in range(NC)], axis=0)  # (B, D)
    series = (sums / S).astype(np.float32)
    snorm = np.linalg.norm(series.astype(np.float64), axis=1)
    snorm_safe = np.where(snorm > 0, snorm, 1.0)

    # ---- L2: sims group-max + top-8 groups per shard, bank-sharded ----
    mbT = mb.T  # (D, M) fp32 view
    bankT8 = (mbT * Q8_SCALE).astype(NP_F8E4)  # (D, M) fp8e4m3, prescaled
    qT8 = np.ascontiguousarray((series.T * Q8_SCALE).astype(NP_F8E4))
    l2 = _get("l2")
    pad = np.zeros((D, MLP - ML), NP_F8E4)
    in_maps = [
        {"qT": qT8,
         "bankT": np.ascontiguousarray(
             np.concatenate([bankT8[:, i * ML:(i + 1) * ML], pad], axis=1))}
        for i in range(NC)
    ]
    r2 = _run(l2, in_maps, "l2")
    tv = np.stack([r2[i]["tv"] for i in range(NC)], axis=0)     # (NC, B, 48)
    gvals = tv[:, :, :8].astype(np.float32) / (Q8_SCALE * Q8_SCALE)

    def _u16(lo, hi):
        return (np.ascontiguousarray(tv[:, :, lo:hi]).view(np.uint16)
                .astype(np.int64))

    # unwind the run-8 chain: a position < 8 at any level points into the
    # previous fold's top-8; otherwise it is an offset into that level's
    # region of the group array.  Fold 0 has no predecessor, so its
    # positions are direct group ids.
    nf = len(L2_STAGES)
    posF = _u16(8, 16)
    fold_pos = [_u16(16 + 8 * (nf - 1 - r), 24 + 8 * (nf - 1 - r))
                for r in range(nf)]                             # r = 0..nf-1
    gidx = np.where(posF >= 8, _SG[nf] + posF - 8, -1)
    carry = np.minimum(posF, 7)
    for r in range(nf - 1, 0, -1):
        p = np.take_along_axis(fold_pos[r], carry, axis=2)
        gidx = np.where((gidx < 0) & (p >= 8), _SG[r] + p - 8, gidx)
        carry = np.where(gidx < 0, np.minimum(p, 7), carry)
    p0 = np.take_along_axis(fold_pos[0], carry, axis=2)
    gidx = np.where(gidx < 0, p0, gidx)
    bad = (gidx >= NG) | (gidx < 0)
    gidx = np.where(bad, 0, gidx)

    # candidate columns: 8 groups x 8 cols per (core, query)
    cols = GROUP_COLS[gidx]                                     # (NC,B,8,8)
    valid = (~bad[:, :, :, None]) & (gidx[:, :, :, None] < NG) & (cols < ML)
    grow = cols + (np.arange(NC, dtype=np.int64) * ML)[:, None, None, None]
    grow = np.where(valid, grow, 0)
    rows_q = grow.transpose(1, 0, 2, 3).reshape(B, NC * 64)     # (B, 512)
    valid_q = valid.transpose(1, 0, 2, 3).reshape(B, NC * 64)

    # exact host rescan of the candidate columns (f32)
    sims_sub = np.empty((B, NC * 64), np.float32)
    CH = 32
    for q0 in range(0, B, CH):
        sl = slice(q0, q0 + CH)
        gathered = mb[rows_q[sl]]                               # (CH, 512, D)
        sims_sub[sl] = np.einsum(
            "qkd,qd->qk", gathered, series[sl], optimize=True)
    cosv = sims_sub / snorm_safe[:, None]
    cosv = np.where(valid_q, cosv, -np.inf)
    cosv = np.where(cosv > 0.999, -np.inf, cosv)               # exclude_self
    cosv = np.where(cosv >= THRESH, cosv, -np.inf)             # threshold

    part = np.argpartition(-cosv, TOPK - 1, axis=1)[:, :TOPK]
    topv = np.take_along_axis(cosv, part, axis=1)              # (B, 16)
    topi = np.take_along_axis(rows_q, part, axis=1)            # (B, 16)
    order = np.argsort(-topv, axis=1, kind="stable")
    topv = np.take_along_axis(topv, order, axis=1)
    topi = np.take_along_axis(topi, order, axis=1)

    # Sufficiency: a shard can only hide a true top-16 element if all 8 of
    # its returned group-maxima beat the merged 16th-best value.  On the
    # (never-observed) failure, recompute that query exactly on host.
    v16 = topv[:, TOPK - 1]                                    # (B,)
    g8min = gvals.min(axis=2) / snorm_safe[None, :]            # (NC, B)
    flagged = np.where((g8min > v16[None, :] - SUFF_MARGIN).any(axis=0))[0]
    for q in flagged:
        cos_all = (mb @ series[q]) / snorm_safe[q]
        cos_all = np.where(cos_all > 0.999, -np.inf, cos_all)
        cos_all = np.where(cos_all >= THRESH, cos_all, -np.inf)
        pq = np.argpartition(-cos_all, TOPK - 1)[:TOPK]
        vq = cos_all[pq]
        oq = np.argsort(-vq, kind="stable")
        topv[q] = vq[oq]
        topi[q] = pq[oq]

    if not np.any(topv > -np.inf):
        # nothing retrieved anywhere -> output == hidden_states exactly
        return hs.copy()

    topv_dev = np.where(np.isfinite(topv), topv, NEG).astype(np.float32)
    # guard: gather index for -inf slots is arbitrary but harmless (masked)
    topi = np.where(np.isfinite(topv), topi, 0)

    # ---- L3: attention + gate + LN + broadcast add, batch-sharded ----
    wgs, wgm = np.ascontiguousarray(Wg[0, :D]), np.ascontiguousarray(Wg[0, D:])
    reps = np.ascontiguousarray(np.stack([bv, bo, wgs, wgm, ln_g, ln_b]))
    l3 = _get("l3")
    # fp8 weights, prescaled by Q8_SCALE (device upconverts with /64)
    wcat = np.concatenate([Wq.T, Wk.T], axis=1).astype(NP_BF16)
    wvwo = np.ascontiguousarray(
        (np.concatenate([Wv.T, Wo.T], axis=1) * Q8_SCALE).astype(NP_F8E4))
    bqbk = np.ascontiguousarray(
        np.concatenate([bq, bk]).reshape(2, KJ, 128)
        .transpose(2, 0, 1).reshape(128, 2 * KJ))
    bg_col = np.full((BL, 1), bg[0], np.float32)
    in_maps = []
    for i in range(NC):
        sl = slice(i * BL, (i + 1) * BL)
        idx_flat = topi[sl].reshape(-1)  # (BL*16,)
        wpack = np.concatenate(
            [wcat, series[sl].T.astype(NP_BF16),
             mbT[:, idx_flat].astype(NP_BF16)], axis=1)
        sm32 = np.concatenate(
            [series[sl], topv_dev[sl], bg_col], axis=1).astype(np.float32)
        in_maps.append({
            "hsT": hsT[sl],
            "wpack": np.ascontiguousarray(wpack),
            "wvwo": wvwo,
            "bqbk": bqbk,
            "sm32": np.ascontiguousarray(sm32),
            "reps": reps,
        })
    r3 = _run(l3, in_maps, "l3")
    outT = np.concatenate([r3[i]["out"] for i in range(NC)], axis=0)  # (B,D,S)
    return outT.transpose(0, 2, 1).astype(np.float32)
